# revision 10
# baseline (speedup 1.0000x reference)
"""Distributed Bass kernel for attention-energy softmax on 8 TRN2 NeuronCores.

Computes: softmax(enc @ W.T @ h + (b.h)) == softmax(enc @ (W.T @ h)) over S=32768.
The bias term b.h is a constant shift across all energies and cancels in softmax,
so b is unused.

Sharding: encoder_output split along S into 8 shards of 4096 rows; each shard is
host-transposed to [H, S_shard] so the contraction dim (H) lands on SBUF
partitions. W and hidden_state are replicated. Each core computes its local
energies + local softmax stats (max, sum-of-exp), an AllGather of the 2-float
stats produces the global max/normalizer, and each core emits its normalized
slice of the attention weights.
"""

import sys

sys.path.insert(0, "/opt/trn_rl_repo")

import numpy as np

import concourse.bacc as bacc
import concourse.mybir as mybir
import concourse.tile as tile
from concourse.bass_utils import run_bass_kernel_spmd

N_CORES = 8
H = 1024
S = 32768
S_SHARD = S // N_CORES          # 4096
HC = H // 128                   # 8 h-chunks of 128 (contraction tiles)
SC = 2                          # s half-slabs per h-chunk (1 MiB DMA slabs)
S_SLAB = S_SHARD // SC          # 2048
N_STILE = S_SHARD // 128        # 32 output columns
FP32 = mybir.dt.float32
RG = [list(range(N_CORES))]

_compiled_nc = None


def _build():
    nc = bacc.Bacc(
        "TRN2", target_bir_lowering=False, debug=False, num_devices=N_CORES
    )

    encT = nc.dram_tensor("encT", [H, S_SHARD], FP32, kind="ExternalInput")
    h2 = nc.dram_tensor("h2", [128, HC], FP32, kind="ExternalInput")
    W = nc.dram_tensor("W", [H, H], FP32, kind="ExternalInput")
    ident = nc.dram_tensor("ident", [128, 128], FP32, kind="ExternalInput")
    out_ext = nc.dram_tensor("out", [128, N_STILE], FP32, kind="ExternalOutput")

    EXP = mybir.ActivationFunctionType.Exp
    AX = mybir.AxisListType.X

    with tile.TileContext(nc) as tc:
        with (
            tc.tile_pool(name="sb", bufs=1) as sb,
            tc.tile_pool(name="enc", bufs=3) as encp,
            tc.tile_pool(name="ps", bufs=1, space="PSUM") as psp,
            tc.tile_pool(name="dram", bufs=1, space="DRAM") as dramp,
        ):
            # --- constants / small inputs ---
            W_sb = sb.tile([128, HC * H], FP32, tag="W")
            h_sb = sb.tile([128, HC], FP32, tag="h")
            ones_r = sb.tile([1, 128], FP32, tag="ones_r")
            ones_c = sb.tile([128, 1], FP32, tag="ones_c")
            id_sb = sb.tile([128, 128], FP32, tag="id")

            nc.sync.dma_start(out=id_sb[:, :], in_=ident[:, :])
            nc.sync.dma_start(out=h_sb[:, :], in_=h2[:, :])
            nc.sync.dma_start(
                out=W_sb[:, :].rearrange("p (c j) -> p c j", c=HC),
                in_=W[:, :].rearrange("(c p) j -> p c j", p=128),
            )
            nc.vector.memset(ones_r[:, :], 1.0)
            nc.vector.memset(ones_c[:, :], 1.0)

            # --- v = W.T @ h  (v_ps[:, jc] = v[jc*128 + p]) ---
            # All MMs into v_ps form ONE accumulation group (one PSUM bank):
            # start=True clears has_written for the whole bank, so per-column
            # groups can't interleave; per-element has_written makes a single
            # group over element-disjoint columns exact.
            v_ps = psp.tile([128, HC], FP32, tag="vps")
            for jc in range(HC):
                for kc in range(HC):
                    o = kc * H + jc * 128
                    nc.tensor.matmul(
                        v_ps[:, jc : jc + 1],
                        lhsT=W_sb[:, o : o + 128],
                        rhs=h_sb[:, kc : kc + 1],
                        start=(jc == 0 and kc == 0),
                        stop=(jc == HC - 1 and kc == HC - 1),
                    )
            v_sb = sb.tile([128, HC], FP32, tag="v")
            nc.vector.tensor_copy(v_sb[:, :], v_ps[:, :])

            # --- e = enc @ v  (e_ps[p, col] = e[col*128 + p]) ---
            e_ps = psp.tile([128, N_STILE], FP32, tag="eps")
            for hc in range(HC):
                for sc in range(SC):
                    slab = encp.tile([128, S_SLAB], FP32, tag="slab")
                    nc.sync.dma_start(
                        out=slab[:, :],
                        in_=encT[
                            hc * 128 : (hc + 1) * 128,
                            sc * S_SLAB : (sc + 1) * S_SLAB,
                        ],
                    )
                    for t in range(S_SLAB // 128):
                        col = sc * (S_SLAB // 128) + t
                        nc.tensor.matmul(
                            e_ps[:, col : col + 1],
                            lhsT=slab[:, t * 128 : (t + 1) * 128],
                            rhs=v_sb[:, hc : hc + 1],
                            start=(hc == 0 and col == 0),
                            stop=(hc == HC - 1 and col == N_STILE - 1),
                        )

            # --- local softmax stats ---
            m_p = sb.tile([128, 1], FP32, tag="mp")
            nc.vector.reduce_max(m_p[:, :], e_ps[:, :], axis=AX)
            # partition-axis max: PE-transpose the per-partition maxes to one
            # row, then reduce along the free dim
            mT_ps = psp.tile([1, 128], FP32, tag="mT")
            nc.tensor.transpose(mT_ps[:, :], m_p[:, :], id_sb[:, :])
            m0 = sb.tile([1, 1], FP32, tag="m0")
            nc.vector.reduce_max(m0[0:1, 0:1], mT_ps[0:1, :], axis=AX)
            # broadcast local max to all partitions via ones-matmul
            mb_ps = psp.tile([128, 1], FP32, tag="mbps")
            nc.tensor.matmul(
                mb_ps[:, :], lhsT=ones_r[:, :], rhs=m0[0:1, 0:1],
                start=True, stop=True,
            )
            neg_m = sb.tile([128, 1], FP32, tag="negm")
            nc.scalar.mul(neg_m[:, :], mb_ps[:, :], -1.0)

            p_vals = sb.tile([128, N_STILE], FP32, tag="pv")
            s_p = sb.tile([128, 1], FP32, tag="sp")
            nc.scalar.activation(
                p_vals[:, :], e_ps[:, :], EXP, bias=neg_m[:, :], scale=1.0,
                accum_out=s_p[:, :],
            )
            # S_loc = sum over partitions of s_p, via ones-matmul
            S_ps = psp.tile([1, 1], FP32, tag="Sps")
            nc.tensor.matmul(
                S_ps[:, :], lhsT=s_p[:, :], rhs=ones_c[:, :], start=True, stop=True
            )

            # --- exchange (m_loc, S_loc) across cores ---
            stats_sb = sb.tile([1, 2], FP32, tag="stats")
            nc.vector.tensor_copy(stats_sb[0:1, 0:1], m0[0:1, 0:1])
            nc.vector.tensor_copy(stats_sb[0:1, 1:2], S_ps[0:1, 0:1])

            stats_d = dramp.tile([1, 2], FP32, tag="statsd")
            gath_d = dramp.tile([N_CORES, 2], FP32, tag="gathd")
            nc.sync.dma_start(out=stats_d[:, :], in_=stats_sb[0:1, :])
            nc.gpsimd.collective_compute(
                "AllGather",
                mybir.AluOpType.bypass,
                replica_groups=RG,
                ins=[stats_d.opt()],
                outs=[gath_d.opt()],
            )
            gath_sb = sb.tile([1, 2 * N_CORES], FP32, tag="gath")
            nc.sync.dma_start(
                out=gath_sb[0:1, :], in_=gath_d[:, :].rearrange("a b -> (a b)")
            )

            # --- global combine (all on partition 0) ---
            M_sb = sb.tile([1, 1], FP32, tag="M")
            negM = sb.tile([1, 1], FP32, tag="negM")
            t8 = sb.tile([1, N_CORES], FP32, tag="t8")
            z8 = sb.tile([1, N_CORES], FP32, tag="z8")
            Z_sb = sb.tile([1, 1], FP32, tag="Z")
            rZ = sb.tile([1, 1], FP32, tag="rZ")
            r_sb = sb.tile([1, 1], FP32, tag="r")
            sc1 = sb.tile([1, 1], FP32, tag="sc1")

            ms = gath_sb[0:1, 0 : 2 * N_CORES : 2]
            ss = gath_sb[0:1, 1 : 2 * N_CORES : 2]
            nc.vector.reduce_max(M_sb[:, :], ms, axis=AX)
            nc.vector.tensor_scalar_mul(negM[:, :], M_sb[:, :], -1.0)
            nc.scalar.activation(t8[0:1, :], ms, EXP, bias=negM[0:1, 0:1])
            nc.vector.tensor_mul(z8[0:1, :], t8[0:1, :], ss)
            nc.vector.reduce_sum(Z_sb[:, :], z8[0:1, :], axis=AX)
            nc.vector.reciprocal(rZ[:, :], Z_sb[:, :])
            nc.scalar.activation(r_sb[:, :], m0[0:1, 0:1], EXP, bias=negM[0:1, 0:1])
            nc.vector.tensor_mul(sc1[:, :], r_sb[:, :], rZ[:, :])

            # broadcast final scale, apply, store
            sc_ps = psp.tile([128, 1], FP32, tag="scps")
            nc.tensor.matmul(
                sc_ps[:, :], lhsT=ones_r[:, :], rhs=sc1[0:1, 0:1],
                start=True, stop=True,
            )
            sc_sb = sb.tile([128, 1], FP32, tag="scsb")
            nc.vector.tensor_copy(sc_sb[:, :], sc_ps[:, :])
            out_sb = sb.tile([128, N_STILE], FP32, tag="outsb")
            nc.vector.tensor_scalar_mul(out_sb[:, :], p_vals[:, :], sc_sb[:, :])
            nc.sync.dma_start(out=out_ext[:, :], in_=out_sb[:, :])

    nc.compile()
    return nc


def get_nc():
    global _compiled_nc
    if _compiled_nc is None:
        _compiled_nc = _build()
    return _compiled_nc


def make_in_maps(hidden_state, encoder_output, W):
    h = np.asarray(hidden_state, dtype=np.float32).reshape(H)
    enc = np.asarray(encoder_output, dtype=np.float32).reshape(S, H)
    Wf = np.ascontiguousarray(np.asarray(W, dtype=np.float32).reshape(H, H))
    h2 = np.ascontiguousarray(h.reshape(HC, 128).T)  # h2[p, c] = h[c*128 + p]
    ident = np.eye(128, dtype=np.float32)
    in_maps = []
    for c in range(N_CORES):
        shard = np.ascontiguousarray(
            enc[c * S_SHARD : (c + 1) * S_SHARD, :].T
        )  # [H, S_SHARD]
        in_maps.append({"encT": shard, "h2": h2, "W": Wf, "ident": ident})
    return in_maps


def unshard(results):
    out = np.empty((1, S), dtype=np.float32)
    for c in range(N_CORES):
        # out_core[p, col] = softmax weight for s_local = col*128 + p
        out[0, c * S_SHARD : (c + 1) * S_SHARD] = (
            results[c]["out"].T.reshape(S_SHARD)
        )
    return out


def kernel(hidden_state, encoder_output, W, b=None, **_unused):
    nc = get_nc()
    in_maps = make_in_maps(hidden_state, encoder_output, W)
    res = run_bass_kernel_spmd(nc, in_maps, core_ids=list(range(N_CORES)))
    return unshard(res.results)


# revision 12
# speedup vs baseline: 1.1068x; 1.1068x over previous
"""Distributed Bass kernel for attention-energy softmax on 8 TRN2 NeuronCores.

Computes: softmax(enc @ W.T @ h + (b.h)) == softmax(enc @ (W.T @ h)) over S=32768.
The bias term b.h is a constant shift across all energies and cancels in softmax,
so b is unused.

Sharding: encoder_output split along S into 8 shards of 4096 rows; each shard is
host-transposed to [H, S_shard] so the contraction dim (H) lands on SBUF
partitions. W and hidden_state are replicated.

Per core:
  v_row = h @ W.T            (h stationary [128,1] chunks, W moving N=512)
  v_bc[hc] = outer(v chunk, ones)  -> [128,128] per h-chunk, v value on all cols
  e_b = sum_hc v_bc[hc].T @ encT_slab   -> PSUM [128,512] per 512-wide s-block,
        energies replicated across partitions (no partition reductions needed)
  flash-style local stats (per-bank max + sum-of-exp, combined), AllGather of
  (m_loc, S_loc), replicated combine via broadcast DMA, final
  out = exp(e - M - ln Z) emitted directly.
"""

import sys

sys.path.insert(0, "/opt/trn_rl_repo")

import numpy as np

import concourse.bacc as bacc
import concourse.mybir as mybir
import concourse.tile as tile
from concourse.bass_utils import run_bass_kernel_spmd

N_CORES = 8
H = 1024
S = 32768
S_SHARD = S // N_CORES          # 4096
HC = H // 128                   # 8 h-chunks of 128 (contraction tiles)
SC = 2                          # s halves (slab DMA granularity: 1 MiB)
S_SLAB = S_SHARD // SC          # 2048
NB = S_SHARD // 512             # 8 PSUM banks of 512 energies
FP32 = mybir.dt.float32
RG = [list(range(N_CORES))]

_compiled_nc = None


def _build():
    nc = bacc.Bacc(
        "TRN2", target_bir_lowering=False, debug=False, num_devices=N_CORES
    )

    encT = nc.dram_tensor("encT", [H, S_SHARD], FP32, kind="ExternalInput")
    h2 = nc.dram_tensor("h2", [128, HC], FP32, kind="ExternalInput")
    W = nc.dram_tensor("W", [H, H], FP32, kind="ExternalInput")
    out_ext = nc.dram_tensor("out", [1, S_SHARD], FP32, kind="ExternalOutput")

    EXP = mybir.ActivationFunctionType.Exp
    LN = mybir.ActivationFunctionType.Ln
    AX = mybir.AxisListType.X

    with tile.TileContext(nc) as tc:
        with (
            tc.tile_pool(name="sb", bufs=1) as sb,
            tc.tile_pool(name="enc", bufs=10) as encp,
            tc.tile_pool(name="dram", bufs=1, space="DRAM") as dramp,
        ):
            # --- constants / small inputs ---
            W_sb = sb.tile([128, HC * H], FP32, tag="W")
            h_sb = sb.tile([128, HC], FP32, tag="h")
            ones_r = sb.tile([1, 128], FP32, tag="ones_r")

            nc.sync.dma_start(out=h_sb[:, :], in_=h2[:, :])
            nc.sync.dma_start(
                out=W_sb[:, :].rearrange("p (c j) -> p c j", c=HC),
                in_=W[:, :].rearrange("(c p) j -> p c j", p=128),
            )
            nc.vector.memset(ones_r[:, :], 1.0)

            # --- v phase: v_row[0, j] = v[j] = sum_k W[k, j] h[k] ---
            v_row_sb = sb.tile([1, H], FP32, tag="vrow")
            v_bc = sb.tile([128, H], FP32, tag="vbc")
            with tc.tile_pool(name="psv", bufs=1, space="PSUM") as psv:
                v_row_ps = psv.tile([1, H], FP32, tag="vrps")
                for jb in range(H // 512):
                    for kc in range(HC):
                        nc.tensor.matmul(
                            v_row_ps[0:1, jb * 512 : (jb + 1) * 512],
                            lhsT=h_sb[:, kc : kc + 1],
                            rhs=W_sb[:, kc * H + jb * 512 : kc * H + jb * 512 + 512],
                            start=(kc == 0),
                            stop=(kc == HC - 1),
                        )
                nc.vector.tensor_copy(v_row_sb[:, :], v_row_ps[:, :])
                # broadcast v chunks onto partitions: v_bc[:, hc*128+n][k] =
                # v[hc*128+k] via outer product with a row of ones
                for hc in range(HC):
                    vb_ps = psv.tile([128, 128], FP32, tag="vb")
                    nc.tensor.matmul(
                        vb_ps[:, :],
                        lhsT=v_row_sb[0:1, hc * 128 : (hc + 1) * 128],
                        rhs=ones_r[0:1, :],
                        start=True,
                        stop=True,
                    )
                    nc.vector.tensor_copy(
                        v_bc[:, hc * 128 : (hc + 1) * 128], vb_ps[:, :]
                    )

            # --- e phase: 8 PSUM banks of [128, 512], replicated rows ---
            mx = sb.tile([128, NB], FP32, tag="mx")
            ngx = sb.tile([128, NB], FP32, tag="ngx")
            s_bank = sb.tile([128, NB], FP32, tag="sbank")
            with tc.tile_pool(name="pse", bufs=1, space="PSUM") as pse:
                e_b = [
                    pse.tile([128, 512], FP32, tag=f"eb{b}", name=f"eb{b}")
                    for b in range(NB)
                ]

                for sc in range(SC):
                    slabs = []
                    for hc in range(HC):
                        slab = encp.tile([128, S_SLAB], FP32, tag="slab")
                        nc.sync.dma_start(
                            out=slab[:, :],
                            in_=encT[
                                hc * 128 : (hc + 1) * 128,
                                sc * S_SLAB : (sc + 1) * S_SLAB,
                            ],
                        )
                        slabs.append(slab)
                    for jb in range(S_SLAB // 512):
                        b = sc * (S_SLAB // 512) + jb
                        for hc in range(HC):
                            nc.tensor.matmul(
                                e_b[b][:, :],
                                lhsT=v_bc[:, hc * 128 : (hc + 1) * 128],
                                rhs=slabs[hc][:, jb * 512 : (jb + 1) * 512],
                                start=(hc == 0),
                                stop=(hc == HC - 1),
                            )
                    # per-bank local stats (overlap with next sc's DMA/MMs)
                    for jb in range(S_SLAB // 512):
                        b = sc * (S_SLAB // 512) + jb
                        nc.vector.reduce_max(
                            mx[:, b : b + 1], e_b[b][:, :], axis=AX
                        )
                        nc.vector.tensor_scalar_mul(
                            ngx[:, b : b + 1], mx[:, b : b + 1], -1.0
                        )
                        scratch = sb.tile([128, 512], FP32, tag="scr")
                        nc.scalar.activation(
                            scratch[:, :], e_b[b][:, :], EXP,
                            bias=ngx[:, b : b + 1], scale=1.0,
                            accum_out=s_bank[:, b : b + 1],
                        )

                # combine per-bank stats -> (m_loc, S_loc); all replicated
                m_loc = sb.tile([128, 1], FP32, tag="mloc")
                ngl = sb.tile([128, 1], FP32, tag="ngl")
                corr = sb.tile([128, NB], FP32, tag="corr")
                sterm = sb.tile([128, NB], FP32, tag="sterm")
                S_loc = sb.tile([128, 1], FP32, tag="Sloc")
                nc.vector.reduce_max(m_loc[:, :], mx[:, :], axis=AX)
                nc.vector.tensor_scalar_mul(ngl[:, :], m_loc[:, :], -1.0)
                nc.scalar.activation(
                    corr[:, :], mx[:, :], EXP, bias=ngl[:, :], scale=1.0
                )
                nc.vector.tensor_mul(sterm[:, :], corr[:, :], s_bank[:, :])
                nc.vector.reduce_sum(S_loc[:, :], sterm[:, :], axis=AX)

                # --- exchange (m_loc, S_loc) across cores ---
                stats_sb = sb.tile([1, 2], FP32, tag="stats")
                nc.vector.tensor_copy(stats_sb[0:1, 0:1], m_loc[0:1, 0:1])
                nc.vector.tensor_copy(stats_sb[0:1, 1:2], S_loc[0:1, 0:1])

                stats_d = dramp.tile([1, 2], FP32, tag="statsd")
                gath_d = dramp.tile([N_CORES, 2], FP32, tag="gathd")
                nc.sync.dma_start(out=stats_d[:, :], in_=stats_sb[0:1, :])
                nc.gpsimd.collective_compute(
                    "AllGather",
                    mybir.AluOpType.bypass,
                    replica_groups=RG,
                    ins=[stats_d.opt()],
                    outs=[gath_d.opt()],
                )
                # broadcast-load gathered stats onto all 128 partitions
                gath_sb = sb.tile([128, 2 * N_CORES], FP32, tag="gath")
                nc.sync.dma_start(
                    out=gath_sb[:, :],
                    in_=gath_d[:, :]
                    .rearrange("a b -> (a b)")
                    .partition_broadcast(128),
                )

                # --- global combine, replicated on all partitions ---
                ms = gath_sb[:, 0 : 2 * N_CORES : 2]
                ss = gath_sb[:, 1 : 2 * N_CORES : 2]
                M_g = sb.tile([128, 1], FP32, tag="Mg")
                ngM = sb.tile([128, 1], FP32, tag="ngM")
                t8 = sb.tile([128, N_CORES], FP32, tag="t8")
                z8 = sb.tile([128, N_CORES], FP32, tag="z8")
                Z_g = sb.tile([128, 1], FP32, tag="Zg")
                lnZ = sb.tile([128, 1], FP32, tag="lnZ")
                bias_f = sb.tile([128, 1], FP32, tag="biasf")
                nc.vector.reduce_max(M_g[:, :], ms, axis=AX)
                nc.vector.tensor_scalar_mul(ngM[:, :], M_g[:, :], -1.0)
                nc.scalar.activation(
                    t8[:, :], ms, EXP, bias=ngM[:, :], scale=1.0
                )
                nc.vector.tensor_mul(z8[:, :], t8[:, :], ss)
                nc.vector.reduce_sum(Z_g[:, :], z8[:, :], axis=AX)
                nc.scalar.activation(lnZ[:, :], Z_g[:, :], LN)
                nc.vector.tensor_add(bias_f[:, :], M_g[:, :], lnZ[:, :])
                nc.vector.tensor_scalar_mul(bias_f[:, :], bias_f[:, :], -1.0)

                # --- final: out = exp(e - M - lnZ), take partition-0 row ---
                out_full = sb.tile([128, S_SHARD], FP32, tag="outf")
                for b in range(NB):
                    nc.scalar.activation(
                        out_full[:, b * 512 : (b + 1) * 512], e_b[b][:, :],
                        EXP, bias=bias_f[:, :], scale=1.0,
                    )
                nc.sync.dma_start(out=out_ext[:, :], in_=out_full[0:1, :])

    nc.compile()
    return nc


def get_nc():
    global _compiled_nc
    if _compiled_nc is None:
        _compiled_nc = _build()
    return _compiled_nc


def make_in_maps(hidden_state, encoder_output, W):
    h = np.asarray(hidden_state, dtype=np.float32).reshape(H)
    enc = np.asarray(encoder_output, dtype=np.float32).reshape(S, H)
    Wf = np.ascontiguousarray(np.asarray(W, dtype=np.float32).reshape(H, H))
    h2 = np.ascontiguousarray(h.reshape(HC, 128).T)  # h2[p, c] = h[c*128 + p]
    in_maps = []
    for c in range(N_CORES):
        shard = np.ascontiguousarray(
            enc[c * S_SHARD : (c + 1) * S_SHARD, :].T
        )  # [H, S_SHARD]
        in_maps.append({"encT": shard, "h2": h2, "W": Wf})
    return in_maps


def unshard(results):
    out = np.empty((1, S), dtype=np.float32)
    for c in range(N_CORES):
        out[0, c * S_SHARD : (c + 1) * S_SHARD] = results[c]["out"].reshape(
            S_SHARD
        )
    return out


def kernel(hidden_state, encoder_output, W, b=None, **_unused):
    nc = get_nc()
    in_maps = make_in_maps(hidden_state, encoder_output, W)
    res = run_bass_kernel_spmd(nc, in_maps, core_ids=list(range(N_CORES)))
    return unshard(res.results)


# revision 13
# speedup vs baseline: 1.5226x; 1.3757x over previous
"""Distributed Bass kernel for attention-energy softmax on 8 TRN2 NeuronCores.

Computes: softmax(enc @ W.T @ h + (b.h)) == softmax(enc @ (W.T @ h)) over S=32768.
The bias term b.h is a constant shift across all energies and cancels in softmax,
so b is unused.

Sharding: encoder_output split along S into 8 shards of 4096 rows; each shard is
host-transposed to [H, S_shard] and cast to fp16 so the contraction dim (H)
lands on SBUF partitions and DMA/TensorE run at 16-bit rates. W and h are
replicated, each split into fp16 (hi, lo) pairs so v = W.T @ h is computed to
~2^-24: v = Wh.T hh + Wh.T hl + Wl.T hh (the dropped Wl.hl term is ~2^-24).
fp16 products accumulate exactly in fp32 PSUM, so the only energy error is the
dropped (enc - fp16(enc)) @ v term, ~2^-12 * |v| ~ 0.008 absolute on energies
of std ~32 (softmax rel err ~1%, well under the 2e-2 gate).

Per core:
  v_row[1,1024] = sum over fp16 term pairs (h chunk stationary, W moving N=512)
  v_col[128,8]  = per-chunk PE transpose of v_row (outer product with [1,1])
  e_b[1,512] x8 = sum_hc {vh,vl}_col[:,hc].T @ enc_slab  (M=1, N=512, fp16)
  flash-style local stats per bank, AllGather of (m_loc, S_loc),
  out = exp(e - m_b) * exp(m_b - M)/Z  via per-bank scalar multiply.
"""

import sys

sys.path.insert(0, "/opt/trn_rl_repo")

import numpy as np

import concourse.bacc as bacc
import concourse.mybir as mybir
import concourse.tile as tile
from concourse.bass_utils import run_bass_kernel_spmd

N_CORES = 8
H = 1024
S = 32768
S_SHARD = S // N_CORES          # 4096
HC = H // 128                   # 8 h-chunks of 128 (contraction tiles)
SC = 2                          # s halves (slab DMA granularity)
S_SLAB = S_SHARD // SC          # 2048
NB = S_SHARD // 512             # 8 PSUM banks of 512 energies
BPS = S_SLAB // 512             # banks per s-half
FP32 = mybir.dt.float32
FP16 = mybir.dt.float16
RG = [list(range(N_CORES))]

_compiled_nc = None


def _build():
    nc = bacc.Bacc(
        "TRN2", target_bir_lowering=False, debug=False, num_devices=N_CORES
    )

    encT = nc.dram_tensor("encT", [H, S_SHARD], FP16, kind="ExternalInput")
    hh2 = nc.dram_tensor("hh2", [128, HC], FP16, kind="ExternalInput")
    hl2 = nc.dram_tensor("hl2", [128, HC], FP16, kind="ExternalInput")
    Wh = nc.dram_tensor("Wh", [H, H], FP16, kind="ExternalInput")
    Wl = nc.dram_tensor("Wl", [H, H], FP16, kind="ExternalInput")
    out_ext = nc.dram_tensor("out", [1, S_SHARD], FP32, kind="ExternalOutput")

    EXP = mybir.ActivationFunctionType.Exp
    AX = mybir.AxisListType.X

    with tile.TileContext(nc) as tc:
        with (
            tc.tile_pool(name="sb", bufs=1) as sb,
            tc.tile_pool(name="enc", bufs=18) as encp,
            tc.tile_pool(name="dram", bufs=1, space="DRAM") as dramp,
        ):
            # --- small inputs / constants ---
            Wh_sb = sb.tile([128, HC * H], FP16, tag="Wh")
            Wl_sb = sb.tile([128, HC * H], FP16, tag="Wl")
            hh_sb = sb.tile([128, HC], FP16, tag="hh")
            hl_sb = sb.tile([128, HC], FP16, tag="hl")
            one1 = sb.tile([1, 1], FP32, tag="one1")

            nc.sync.dma_start(out=hh_sb[:, :], in_=hh2[:, :])
            nc.sync.dma_start(out=hl_sb[:, :], in_=hl2[:, :])
            nc.sync.dma_start(
                out=Wh_sb[:, :].rearrange("p (c j) -> p c j", c=HC),
                in_=Wh[:, :].rearrange("(c p) j -> p c j", p=128),
            )
            nc.sync.dma_start(
                out=Wl_sb[:, :].rearrange("p (c j) -> p c j", c=HC),
                in_=Wl[:, :].rearrange("(c p) j -> p c j", p=128),
            )
            nc.vector.memset(one1[:, :], 1.0)

            # --- v phase: v_row[0, j] = v[j] = sum_k W[k, j] h[k] ---
            # fp16 3-term: Wh.hh + Wh.hl + Wl.hh (Wl.hl dropped, ~2^-24)
            v_row_sb = sb.tile([1, H], FP32, tag="vrow")
            v_col = sb.tile([128, HC], FP32, tag="vcol")
            vh_col = sb.tile([128, HC], FP16, tag="vhcol")
            vb_col = sb.tile([128, HC], FP32, tag="vbcol")
            vl_f32 = sb.tile([128, HC], FP32, tag="vlf")
            vl_col = sb.tile([128, HC], FP16, tag="vlcol")
            with tc.tile_pool(name="psv", bufs=1, space="PSUM") as psv:
                v_row_ps = psv.tile([1, H], FP32, tag="vrps")
                terms = [(hh_sb, Wh_sb), (hl_sb, Wh_sb), (hh_sb, Wl_sb)]
                for jb in range(H // 512):
                    for kc in range(HC):
                        for ti, (hx, Wx) in enumerate(terms):
                            nc.tensor.matmul(
                                v_row_ps[0:1, jb * 512 : (jb + 1) * 512],
                                lhsT=hx[:, kc : kc + 1],
                                rhs=Wx[
                                    :, kc * H + jb * 512 : kc * H + jb * 512 + 512
                                ],
                                start=(kc == 0 and ti == 0),
                                stop=(kc == HC - 1 and ti == len(terms) - 1),
                            )
                nc.vector.tensor_copy(v_row_sb[:, :], v_row_ps[:, :])
                # transpose v chunks onto partitions via outer product w/ [1,1]
                v_col_ps = psv.tile([128, HC], FP32, tag="vcps")
                for hc in range(HC):
                    nc.tensor.matmul(
                        v_col_ps[:, hc : hc + 1],
                        lhsT=v_row_sb[0:1, hc * 128 : (hc + 1) * 128],
                        rhs=one1[0:1, 0:1],
                        start=True,
                        stop=True,
                    )
                nc.vector.tensor_copy(v_col[:, :], v_col_ps[:, :])
            # split v into fp16 (hi, lo)
            nc.vector.tensor_copy(vh_col[:, :], v_col[:, :])
            nc.vector.tensor_copy(vb_col[:, :], vh_col[:, :])
            nc.vector.tensor_sub(vl_f32[:, :], v_col[:, :], vb_col[:, :])
            nc.vector.tensor_copy(vl_col[:, :], vl_f32[:, :])

            # --- e phase: 8 PSUM rows [1, 512], M=1 matmuls ---
            mx = sb.tile([1, NB], FP32, tag="mx")
            ngx = sb.tile([1, NB], FP32, tag="ngx")
            s8 = sb.tile([1, NB], FP32, tag="s8")
            scratch = sb.tile([1, S_SHARD], FP32, tag="scr")
            with tc.tile_pool(name="pse", bufs=1, space="PSUM") as pse:
                e_b = [
                    pse.tile([1, 512], FP32, tag=f"eb{b}", name=f"eb{b}")
                    for b in range(NB)
                ]
                for sc in range(SC):
                    slabs = []
                    for hc in range(HC):
                        slab = encp.tile([128, S_SLAB], FP16, tag="slab")
                        nc.sync.dma_start(
                            out=slab[:, :],
                            in_=encT[
                                hc * 128 : (hc + 1) * 128,
                                sc * S_SLAB : (sc + 1) * S_SLAB,
                            ],
                        )
                        slabs.append(slab)
                    for hc in range(HC):
                        for ti, vx in enumerate((vh_col, vl_col)):
                            for jb in range(BPS):
                                b = sc * BPS + jb
                                nc.tensor.matmul(
                                    e_b[b][0:1, :],
                                    lhsT=vx[:, hc : hc + 1],
                                    rhs=slabs[hc][:, jb * 512 : (jb + 1) * 512],
                                    start=(hc == 0 and ti == 0),
                                    stop=(hc == HC - 1 and ti == 1),
                                )
                    # per-bank local stats (overlap with next s-half)
                    for jb in range(BPS):
                        b = sc * BPS + jb
                        nc.vector.reduce_max(
                            mx[0:1, b : b + 1], e_b[b][0:1, :], axis=AX
                        )
                        nc.vector.tensor_scalar_mul(
                            ngx[0:1, b : b + 1], mx[0:1, b : b + 1], -1.0
                        )
                        nc.scalar.activation(
                            scratch[0:1, b * 512 : (b + 1) * 512],
                            e_b[b][0:1, :], EXP,
                            bias=ngx[0:1, b : b + 1], scale=1.0,
                            accum_out=s8[0:1, b : b + 1],
                        )

            # --- local combine: m_loc, S_loc = sum_b s_b exp(mx_b - m_loc) ---
            m_loc = sb.tile([1, 1], FP32, tag="mloc")
            ngl = sb.tile([1, 1], FP32, tag="ngl")
            corr = sb.tile([1, NB], FP32, tag="corr")
            sterm = sb.tile([1, NB], FP32, tag="sterm")
            S_loc = sb.tile([1, 1], FP32, tag="Sloc")
            nc.vector.reduce_max(m_loc[:, :], mx[0:1, :], axis=AX)
            nc.vector.tensor_scalar_mul(ngl[:, :], m_loc[:, :], -1.0)
            nc.scalar.activation(
                corr[0:1, :], mx[0:1, :], EXP, bias=ngl[0:1, 0:1], scale=1.0
            )
            nc.vector.tensor_mul(sterm[0:1, :], corr[0:1, :], s8[0:1, :])
            nc.vector.reduce_sum(S_loc[:, :], sterm[0:1, :], axis=AX)

            # --- exchange (m_loc, S_loc) across cores ---
            stats_sb = sb.tile([1, 2], FP32, tag="stats")
            nc.vector.tensor_copy(stats_sb[0:1, 0:1], m_loc[0:1, 0:1])
            nc.vector.tensor_copy(stats_sb[0:1, 1:2], S_loc[0:1, 0:1])

            stats_d = dramp.tile([1, 2], FP32, tag="statsd")
            gath_d = dramp.tile([N_CORES, 2], FP32, tag="gathd")
            nc.sync.dma_start(out=stats_d[:, :], in_=stats_sb[0:1, :])
            nc.gpsimd.collective_compute(
                "AllGather",
                mybir.AluOpType.bypass,
                replica_groups=RG,
                ins=[stats_d.opt()],
                outs=[gath_d.opt()],
            )
            gath_sb = sb.tile([1, 2 * N_CORES], FP32, tag="gath")
            nc.sync.dma_start(
                out=gath_sb[0:1, :], in_=gath_d[:, :].rearrange("a b -> (a b)")
            )

            # --- global combine on partition 0 ---
            ms = gath_sb[0:1, 0 : 2 * N_CORES : 2]
            ss = gath_sb[0:1, 1 : 2 * N_CORES : 2]
            M_g = sb.tile([1, 1], FP32, tag="Mg")
            ngM = sb.tile([1, 1], FP32, tag="ngM")
            t8 = sb.tile([1, N_CORES], FP32, tag="t8")
            z8 = sb.tile([1, N_CORES], FP32, tag="z8")
            Z_g = sb.tile([1, 1], FP32, tag="Zg")
            rZ = sb.tile([1, 1], FP32, tag="rZ")
            f8 = sb.tile([1, NB], FP32, tag="f8")
            nc.vector.reduce_max(M_g[:, :], ms, axis=AX)
            nc.vector.tensor_scalar_mul(ngM[:, :], M_g[:, :], -1.0)
            nc.scalar.activation(t8[0:1, :], ms, EXP, bias=ngM[0:1, 0:1])
            nc.vector.tensor_mul(z8[0:1, :], t8[0:1, :], ss)
            nc.vector.reduce_sum(Z_g[:, :], z8[0:1, :], axis=AX)
            nc.vector.reciprocal(rZ[:, :], Z_g[:, :])
            # per-bank final factor: f8[b] = exp(mx_b - M) / Z
            nc.scalar.activation(f8[0:1, :], mx[0:1, :], EXP, bias=ngM[0:1, 0:1])
            nc.vector.tensor_scalar_mul(f8[0:1, :], f8[0:1, :], rZ[0:1, 0:1])

            # --- final scale + store ---
            out_row = sb.tile([1, S_SHARD], FP32, tag="outr")
            for b in range(NB):
                nc.vector.tensor_scalar_mul(
                    out_row[0:1, b * 512 : (b + 1) * 512],
                    scratch[0:1, b * 512 : (b + 1) * 512],
                    f8[0:1, b : b + 1],
                )
            nc.sync.dma_start(out=out_ext[:, :], in_=out_row[0:1, :])

    nc.compile()
    return nc


def get_nc():
    global _compiled_nc
    if _compiled_nc is None:
        _compiled_nc = _build()
    return _compiled_nc


def make_in_maps(hidden_state, encoder_output, W):
    h = np.asarray(hidden_state, dtype=np.float32).reshape(H)
    enc = np.asarray(encoder_output, dtype=np.float32).reshape(S, H)
    Wf = np.asarray(W, dtype=np.float32).reshape(H, H)

    h2 = h.reshape(HC, 128).T  # h2[p, c] = h[c*128 + p]
    hh2 = np.ascontiguousarray(h2.astype(np.float16))
    hl2 = np.ascontiguousarray((h2 - hh2.astype(np.float32)).astype(np.float16))
    Wh = np.ascontiguousarray(Wf.astype(np.float16))
    Wl = np.ascontiguousarray((Wf - Wh.astype(np.float32)).astype(np.float16))

    in_maps = []
    for c in range(N_CORES):
        shard = np.ascontiguousarray(
            enc[c * S_SHARD : (c + 1) * S_SHARD, :].T.astype(np.float16)
        )  # [H, S_SHARD] fp16
        in_maps.append(
            {"encT": shard, "hh2": hh2, "hl2": hl2, "Wh": Wh, "Wl": Wl}
        )
    return in_maps


def unshard(results):
    out = np.empty((1, S), dtype=np.float32)
    for c in range(N_CORES):
        out[0, c * S_SHARD : (c + 1) * S_SHARD] = results[c]["out"].reshape(
            S_SHARD
        )
    return out


def kernel(hidden_state, encoder_output, W, b=None, **_unused):
    nc = get_nc()
    in_maps = make_in_maps(hidden_state, encoder_output, W)
    res = run_bass_kernel_spmd(nc, in_maps, core_ids=list(range(N_CORES)))
    return unshard(res.results)


# revision 17
# speedup vs baseline: 2.0508x; 1.3469x over previous
"""Distributed Bass kernel for attention-energy softmax on 8 TRN2 NeuronCores.

Computes: softmax(enc @ W.T @ h + (b.h)) == softmax(enc @ (W.T @ h)) over S=32768.
The bias term b.h is a constant shift across all energies and cancels in softmax,
so b is unused.

Sharding: encoder_output split along S into 8 shards of 4096 rows; each shard is
host-transposed to [H, S_shard] and cast to fp16 so the contraction dim (H)
lands on SBUF partitions and DMA/TensorE run at 16-bit rates. W and h are
replicated fp16. fp16 products accumulate exactly in fp32 PSUM; the softmax
rel err of the fp16 path is ~6e-3 (measured), well under the 2e-2 gate.

Per core:
  v_row[1,1024] = hh.T-stationary @ Wh (moving, N=512)     16 matmuls
  v_col[128,8]  = per-chunk PE transpose of v_row (outer product with [1,1])
  e_b[1,512] x8 = sum_hc vh_col[:,hc].T @ enc_slab (M=1, N=512, fp16) 64 matmuls
  flash-style local stats per bank (negated maxes as ready-to-use exp biases),
  AllGather of (-m_loc, S_loc), out = exp(e - m_b) * exp(m_b - M)/Z.
"""

import sys

sys.path.insert(0, "/opt/trn_rl_repo")

import numpy as np

import concourse.bacc as bacc
import concourse.mybir as mybir
import concourse.tile as tile
from concourse.bass_utils import run_bass_kernel_spmd

N_CORES = 8
H = 1024
S = 32768
S_SHARD = S // N_CORES          # 4096
HC = H // 128                   # 8 h-chunks of 128 (contraction tiles)
SC = 2                          # s halves (slab DMA granularity)
S_SLAB = S_SHARD // SC          # 2048
NB = S_SHARD // 512             # 8 PSUM banks of 512 energies
BPS = S_SLAB // 512             # banks per s-half
FP32 = mybir.dt.float32
FP16 = mybir.dt.float16
RG = [list(range(N_CORES))]

_compiled_nc = None


def _build():
    nc = bacc.Bacc(
        "TRN2", target_bir_lowering=False, debug=False, num_devices=N_CORES
    )

    encT = nc.dram_tensor("encT", [H, S_SHARD], FP16, kind="ExternalInput")
    hh2 = nc.dram_tensor("hh2", [128, HC], FP16, kind="ExternalInput")
    Wh = nc.dram_tensor("Wh", [H, H], FP16, kind="ExternalInput")
    out_ext = nc.dram_tensor("out", [1, S_SHARD], FP32, kind="ExternalOutput")

    EXP = mybir.ActivationFunctionType.Exp
    AX = mybir.AxisListType.X
    MIN = mybir.AluOpType.min
    MULT = mybir.AluOpType.mult
    ADD = mybir.AluOpType.add

    with tile.TileContext(nc) as tc:
        with (
            tc.tile_pool(name="sb", bufs=1) as sb,
            tc.tile_pool(name="enc", bufs=18) as encp,
            tc.tile_pool(name="dram", bufs=1, space="DRAM") as dramp,
        ):
            # --- small inputs / constants ---
            Wh_sb = sb.tile([128, HC * H], FP16, tag="Wh")
            hh_sb = sb.tile([128, HC], FP16, tag="hh")
            one1 = sb.tile([1, 1], FP32, tag="one1")

            nc.sync.dma_start(out=hh_sb[:, :], in_=hh2[:, :])
            nc.sync.dma_start(
                out=Wh_sb[:, :].rearrange("p (c j) -> p c j", c=HC),
                in_=Wh[:, :].rearrange("(c p) j -> p c j", p=128),
            )
            nc.vector.memset(one1[:, :], 1.0)

            # --- v phase: v_row[0, j] = v[j] = sum_k W[k, j] h[k] ---
            v_row_sb = sb.tile([1, H], FP32, tag="vrow")
            v_col = sb.tile([128, HC], FP32, tag="vcol")
            vh_col = sb.tile([128, HC], FP16, tag="vhcol")
            with tc.tile_pool(name="psv", bufs=1, space="PSUM") as psv:
                v_row_ps = psv.tile([1, H], FP32, tag="vrps")
                for jb in range(H // 512):
                    for kc in range(HC):
                        nc.tensor.matmul(
                            v_row_ps[0:1, jb * 512 : (jb + 1) * 512],
                            lhsT=hh_sb[:, kc : kc + 1],
                            rhs=Wh_sb[
                                :, kc * H + jb * 512 : kc * H + jb * 512 + 512
                            ],
                            start=(kc == 0),
                            stop=(kc == HC - 1),
                        )
                nc.vector.tensor_copy(v_row_sb[:, :], v_row_ps[:, :])
                # transpose v chunks onto partitions via outer product w/ [1,1]
                v_col_ps = psv.tile([128, HC], FP32, tag="vcps")
                for hc in range(HC):
                    nc.tensor.matmul(
                        v_col_ps[:, hc : hc + 1],
                        lhsT=v_row_sb[0:1, hc * 128 : (hc + 1) * 128],
                        rhs=one1[0:1, 0:1],
                        start=True,
                        stop=True,
                    )
                nc.vector.tensor_copy(v_col[:, :], v_col_ps[:, :])
            nc.vector.tensor_copy(vh_col[:, :], v_col[:, :])  # cast to fp16

            # --- e phase: 8 PSUM rows [1, 512], M=1 matmuls ---
            mx = sb.tile([1, NB], FP32, tag="mx")
            ngx = sb.tile([1, NB], FP32, tag="ngx")
            s8 = sb.tile([1, NB], FP32, tag="s8")
            scratch = sb.tile([1, S_SHARD], FP32, tag="scr")
            with tc.tile_pool(name="pse", bufs=1, space="PSUM") as pse:
                e_b = [
                    pse.tile([1, 512], FP32, tag=f"eb{b}", name=f"eb{b}")
                    for b in range(NB)
                ]
                for sc in range(SC):
                    slabs = []
                    for hc in range(HC):
                        slab = encp.tile([128, S_SLAB], FP16, tag="slab")
                        nc.sync.dma_start(
                            out=slab[:, :],
                            in_=encT[
                                hc * 128 : (hc + 1) * 128,
                                sc * S_SLAB : (sc + 1) * S_SLAB,
                            ],
                        )
                        slabs.append(slab)
                    for hc in range(HC):
                        for jb in range(BPS):
                            b = sc * BPS + jb
                            nc.tensor.matmul(
                                e_b[b][0:1, :],
                                lhsT=vh_col[:, hc : hc + 1],
                                rhs=slabs[hc][:, jb * 512 : (jb + 1) * 512],
                                start=(hc == 0),
                                stop=(hc == HC - 1),
                            )
                    # per-bank local stats (overlap with next s-half)
                    for jb in range(BPS):
                        b = sc * BPS + jb
                        nc.vector.reduce_max(
                            mx[0:1, b : b + 1], e_b[b][0:1, :], axis=AX
                        )
                        nc.vector.tensor_scalar_mul(
                            ngx[0:1, b : b + 1], mx[0:1, b : b + 1], -1.0
                        )
                        nc.scalar.activation(
                            scratch[0:1, b * 512 : (b + 1) * 512],
                            e_b[b][0:1, :], EXP,
                            bias=ngx[0:1, b : b + 1], scale=1.0,
                            accum_out=s8[0:1, b : b + 1],
                        )

            # --- local combine: S_loc = sum_b s_b exp(mx_b - m_loc) ---
            m_loc = sb.tile([1, 1], FP32, tag="mloc")
            ngl = sb.tile([1, 1], FP32, tag="ngl")
            corr = sb.tile([1, NB], FP32, tag="corr")
            sterm = sb.tile([1, NB], FP32, tag="sterm")
            S_loc = sb.tile([1, 1], FP32, tag="Sloc")
            nc.vector.reduce_max(m_loc[:, :], mx[0:1, :], axis=AX)
            nc.vector.tensor_scalar_mul(ngl[:, :], m_loc[:, :], -1.0)
            nc.scalar.activation(
                corr[0:1, :], mx[0:1, :], EXP, bias=ngl[0:1, 0:1], scale=1.0
            )
            nc.vector.tensor_mul(sterm[0:1, :], corr[0:1, :], s8[0:1, :])
            nc.vector.reduce_sum(S_loc[:, :], sterm[0:1, :], axis=AX)

            # --- exchange (m_loc, S_loc) across cores ---
            stats_sb = sb.tile([1, 2], FP32, tag="stats")
            nc.vector.tensor_copy(stats_sb[0:1, 0:1], m_loc[0:1, 0:1])
            nc.vector.tensor_copy(stats_sb[0:1, 1:2], S_loc[0:1, 0:1])

            stats_d = dramp.tile([1, 2], FP32, tag="statsd")
            gath_d = dramp.tile([N_CORES, 2], FP32, tag="gathd")
            nc.sync.dma_start(out=stats_d[:, :], in_=stats_sb[0:1, :])
            nc.gpsimd.collective_compute(
                "AllGather",
                mybir.AluOpType.bypass,
                replica_groups=RG,
                ins=[stats_d.opt()],
                outs=[gath_d.opt()],
            )
            gath_sb = sb.tile([1, 2 * N_CORES], FP32, tag="gath")
            nc.sync.dma_start(
                out=gath_sb[0:1, :], in_=gath_d[:, :].rearrange("a b -> (a b)")
            )

            # --- global combine on partition 0 ---
            ms = gath_sb[0:1, 0 : 2 * N_CORES : 2]
            ss = gath_sb[0:1, 1 : 2 * N_CORES : 2]
            M_g = sb.tile([1, 1], FP32, tag="Mg")
            ngM = sb.tile([1, 1], FP32, tag="ngM")
            t8 = sb.tile([1, N_CORES], FP32, tag="t8")
            z8 = sb.tile([1, N_CORES], FP32, tag="z8")
            Z_g = sb.tile([1, 1], FP32, tag="Zg")
            rZ = sb.tile([1, 1], FP32, tag="rZ")
            f8 = sb.tile([1, NB], FP32, tag="f8")
            nc.vector.reduce_max(M_g[:, :], ms, axis=AX)
            nc.vector.tensor_scalar_mul(ngM[:, :], M_g[:, :], -1.0)
            nc.scalar.activation(t8[0:1, :], ms, EXP, bias=ngM[0:1, 0:1])
            nc.vector.tensor_mul(z8[0:1, :], t8[0:1, :], ss)
            nc.vector.reduce_sum(Z_g[:, :], z8[0:1, :], axis=AX)
            nc.vector.reciprocal(rZ[:, :], Z_g[:, :])
            # f8[b] = exp(mx_b - M) / Z
            nc.scalar.activation(f8[0:1, :], mx[0:1, :], EXP,
                                 bias=ngM[0:1, 0:1])
            nc.vector.tensor_scalar_mul(f8[0:1, :], f8[0:1, :], rZ[0:1, 0:1])

            # --- final scale + store ---
            out_row = sb.tile([1, S_SHARD], FP32, tag="outr")
            for b in range(NB):
                nc.vector.tensor_scalar_mul(
                    out_row[0:1, b * 512 : (b + 1) * 512],
                    scratch[0:1, b * 512 : (b + 1) * 512],
                    f8[0:1, b : b + 1],
                )
            nc.sync.dma_start(out=out_ext[:, :], in_=out_row[0:1, :])

    nc.compile()
    return nc


def get_nc():
    global _compiled_nc
    if _compiled_nc is None:
        _compiled_nc = _build()
    return _compiled_nc


def make_in_maps(hidden_state, encoder_output, W):
    h = np.asarray(hidden_state, dtype=np.float32).reshape(H)
    enc = np.asarray(encoder_output, dtype=np.float32).reshape(S, H)
    Wf = np.asarray(W, dtype=np.float32).reshape(H, H)

    h2 = h.reshape(HC, 128).T  # h2[p, c] = h[c*128 + p]
    hh2 = np.ascontiguousarray(h2.astype(np.float16))
    Wh = np.ascontiguousarray(Wf.astype(np.float16))

    in_maps = []
    for c in range(N_CORES):
        shard = np.ascontiguousarray(
            enc[c * S_SHARD : (c + 1) * S_SHARD, :].T.astype(np.float16)
        )  # [H, S_SHARD] fp16
        in_maps.append({"encT": shard, "hh2": hh2, "Wh": Wh})
    return in_maps


def unshard(results):
    out = np.empty((1, S), dtype=np.float32)
    for c in range(N_CORES):
        out[0, c * S_SHARD : (c + 1) * S_SHARD] = results[c]["out"].reshape(
            S_SHARD
        )
    return out


def kernel(hidden_state, encoder_output, W, b=None, **_unused):
    nc = get_nc()
    in_maps = make_in_maps(hidden_state, encoder_output, W)
    res = run_bass_kernel_spmd(nc, in_maps, core_ids=list(range(N_CORES)))
    return unshard(res.results)
